# revision 47
# baseline (speedup 1.0000x reference)
"""Trainium2 Bass kernel for nn_DisjointSTModel (GNN message passing + GRU).

Algorithm refactoring (validated vs reference in numpy):
  - A_hat = S @ A_raw @ S with S = diag(1/sqrt(max(deg,1))): normalization is
    separable, so SpMM hops become pure gather-sums with per-node scaling.
  - ELL gather-sum: nodes degree-sorted, chunked; each chunk processed as
    up-to-CLEV levels per dma_gather (4096 idxs, multi-packet, round-robined
    over 4 SWDGE queues for ~3x DMA-engine parallelism).
  - Layer 2 in Horner form: out2 = P0 + S A_raw [S P1 + S^2 A_raw (S P2)],
    with Pk = Hin @ W2_k computed on the tensor engine (f16) once.
  - Layer-1 hops node-sharded over the 8 cores with DVE accumulation;
    layer-2 hops column-sharded (2 of 16 (b,t)-columns per core, f16 256B
    rows) with PE identity-matmul accumulation into per-chunk PSUM banks.
  - GRU feature-major (weights stationary on the PE, f16): all gates via one
    ACT Tanh table (sigmoid algebraically rewritten), x+h gate parts
    accumulated in PSUM, f16 hidden state for 2x DVE throughput.

Launches: A0 (hop1) -> A1 (hop2) -> A (Hin + layer-2) -> B (GRU + head).
Host work between launches is only slicing/concatenation of slices.
"""
import numpy as np

import concourse.bacc as bacc
import concourse.mybir as mybir
import concourse.tile as tile
from concourse.bass_utils import run_bass_kernel_spmd
from concourse.library_config import mlp

P = 128
N = 10000
B = 2
T = 8
M = B * T          # 16 (b,t) columns
HID = 64
NP = 10240         # padded node count (80 cols of 128)
NCOL = NP // P     # 80
NCORES = 8
ZERO_ROW = NP - 1
LPC = NP // NCORES // P   # local cols per core for node-sharded phases = 10

CCHUNK = 512       # col-sharded ELL chunk (global ranks)
CLEV = 8           # levels per gather (num_idxs <= 4096)
NCHUNK = 128       # node-sharded ELL chunk (local positions)
NLEV = 32          # levels per gather (num_idxs <= 4096)
QN = 4             # SWDGE queues; round-robin gathers across them

F32 = mybir.dt.float32
BF16 = mybir.dt.bfloat16
F16 = mybir.dt.float16
I16 = mybir.dt.int16
ADD = mybir.AluOpType.add
MULT = mybir.AluOpType.mult
AFT = mybir.ActivationFunctionType

_CACHE = {}
TIMING_REPS = 1      # >1: wrap each launch body in a repeat loop (for timing)
LAST_WALLS = {}      # launch name -> wall seconds of the spmd call



# ----------------------------------------------------------------------------
# host-side graph preprocessing
# ----------------------------------------------------------------------------

def _pack_idxs(idx_flat):
    t = idx_flat.astype(np.int16).reshape(-1, 16).T
    return np.ascontiguousarray(np.tile(t, (8, 1)))


def _ell_table(deg, offs, src_sorted, node_of, chunk, maxlev, forced_levels=None):
    """Build an ELL gather table for output rows node_of[0..len), chunked by
    `chunk` rows.  Returns (idx_flat, gathers, levels_per_chunk).
    gathers: list of (chunk_j, col16, num_idxs, lv)."""
    n = len(node_of)
    assert n % chunk == 0
    nchunk = n // chunk
    idx_parts, gathers, levels = [], [], []
    col = 0
    for j in range(nchunk):
        nodes = node_of[j * chunk:(j + 1) * chunk]
        dj = int(deg[nodes].max())
        if forced_levels is not None:
            dj = forced_levels[j]
        dj = max(dj, 1)
        levels.append(dj)
        lvl = np.arange(dj)[:, None]
        pos = offs[nodes][None, :] + lvl
        valid = lvl < deg[nodes][None, :]
        slot = np.where(valid, src_sorted[np.minimum(pos, len(src_sorted) - 1)],
                        ZERO_ROW)
        for a in range(0, dj, maxlev):
            lv = min(maxlev, dj - a)
            idx_parts.append(slot[a:a + lv].reshape(-1))
            gathers.append((j, col, lv * chunk, lv))
            col += lv * chunk // 16
    return np.concatenate(idx_parts), gathers, levels


def _prep(edge_src, edge_dst):
    deg_f = np.zeros(N, np.float32)
    np.add.at(deg_f, edge_dst, 1.0)
    dinv = (1.0 / np.sqrt(np.maximum(deg_f, 1.0))).astype(np.float32)

    order = np.argsort(-deg_f, kind="stable")     # old ids in new-rank order
    perm = np.empty(N, np.int64)
    perm[order] = np.arange(N)

    src_n = perm[edge_src]
    dst_n = perm[edge_dst]
    deg_n = np.zeros(NP, np.int64)
    np.add.at(deg_n, dst_n, 1)
    dinv_n = np.ones(NP, np.float32)
    dinv_n[:N] = dinv[order]

    o = np.argsort(dst_n, kind="stable")
    src_sorted = src_n[o].astype(np.int64)
    offs = np.zeros(NP + 1, np.int64)
    np.cumsum(deg_n, out=offs[1:])

    # column-sharded table (global rank order): per-chunk gathers of up to
    # CLEV levels each (so PSUM accumulation per chunk is possible).
    cidx, cgath, clev = _ell_table(deg_n, offs, src_sorted, np.arange(NP),
                                   CCHUNK, CLEV)

    # node-sharded tables: rank i -> core i%8, local pos i//8.
    # levels forced to the max over cores so all 8 cores share one program.
    per_core_nodes = [np.arange(NP)[c::NCORES] for c in range(NCORES)]
    nlev_chunks = NP // NCORES // NCHUNK
    forced = []
    for j in range(nlev_chunks):
        forced.append(max(
            max(int(deg_n[per_core_nodes[c][j * NCHUNK:(j + 1) * NCHUNK]].max()), 1)
            for c in range(NCORES)))
    nidx, ngath = [], None
    for c in range(NCORES):
        fi, fg, _ = _ell_table(deg_n, offs, src_sorted, per_core_nodes[c],
                               NCHUNK, NLEV, forced_levels=forced)
        nidx.append(_pack_idxs(fi))
        ngath = fg

    return dict(order=order, perm=perm, dinv_n=dinv_n,
                cidx=_pack_idxs(cidx), cgath=cgath, clev=clev,
                nidx=nidx, ngath=ngath,
                node_of=per_core_nodes)


# ----------------------------------------------------------------------------
# device programs
# ----------------------------------------------------------------------------

def _wrap(v):
    """[NP] -> [128, NCOL] node-major wrap (n = col*128 + p)."""
    return np.ascontiguousarray(v.reshape(NCOL, P).T)


def _hop_gathers(nc, pool, gbuf, src_d, idx_sb, gathers, acc, elem, width,
                 dt, chunk_cols, ctr):
    """Emit gathers + accumulating adds: acc[:, j*cc:(j+1)*cc, :width] += sum."""
    for (j, c0, nidx, lv) in gathers:
        G = gbuf.tile([P, NLEV, elem], dt, tag="G")
        g = G[:, : nidx // P, :]
        nc.gpsimd.dma_gather(g, src_d[:], idx_sb[:, c0:c0 + nidx // 16],
                             nidx, nidx, elem, single_packet=False,
                             queue_num=ctr[0] % QN)
        ctr[0] += 1
        cc = chunk_cols
        per_lev = nidx // P // lv
        for l in range(lv):
            a = acc[:, j * cc:(j + 1) * cc, :]
            nc.vector.tensor_tensor(
                out=a, in0=a, in1=g[:, l * per_lev:(l + 1) * per_lev, :],
                op=ADD)


def _hop_gathers_psum(nc, gbuf, psH, src_d, idx_sb, gathers, levels, acc,
                      identb, ctr):
    """Col-hop: per-chunk gathers; accumulate levels on the PE via identity
    matmuls into one PSUM tile per chunk, then ACT-copy to acc."""
    cc = CCHUNK // P   # 4 col-blocks of 128 per chunk
    cur_j = -1
    ps = None
    done = 0
    for (j, c0, nidx, lv) in gathers:
        G = gbuf.tile([P, CLEV * cc, P], F16, tag="G")
        g = G[:, : nidx // P, :]
        nc.gpsimd.dma_gather(g, src_d[:], idx_sb[:, c0:c0 + nidx // 16],
                             nidx, nidx, P, single_packet=False,
                             queue_num=ctr[0] % QN)
        ctr[0] += 1
        if j != cur_j:
            cur_j, done = j, 0
            ps = psH.tile([P, cc, P], F32, tag="psH")
        for l in range(lv):
            nc.tensor.matmul(
                ps[:].rearrange("p c f -> p (c f)"),
                identb[:],
                g[:, l * cc:(l + 1) * cc, :].rearrange("p c f -> p (c f)"),
                start=(done == 0), stop=(done == levels[j] - 1))
            done += 1
            if done == levels[j]:
                nc.scalar.copy(out=acc[:, j * cc:(j + 1) * cc, :], in_=ps[:])


def _build_hop_node(tables, first, reps=1):
    """A0 (first=True): xs = x*dinv; hop1 -> z1t_s, z1s_s.
       A1 (first=False): hop2 over z1s -> z2t_s."""
    nc = bacc.Bacc(None, target_bir_lowering=False)
    idx_cols = tables["nidx"][0].shape[1]
    idx_d = nc.dram_tensor("idxn", [P, idx_cols], I16, kind="ExternalInput")
    dinv_d = nc.dram_tensor("dinvl", [P, LPC], F32, kind="ExternalInput")
    dinv2_d = nc.dram_tensor("dinv2l", [P, LPC], F32, kind="ExternalInput")

    if first:
        x_d = nc.dram_tensor("x16", [NP, 16], F32, kind="ExternalInput")
        dinvw_d = nc.dram_tensor("dinvw", [P, NCOL], F32, kind="ExternalInput")
        zt_d = nc.dram_tensor("z1t_s", [NP // NCORES, 16], F32, kind="ExternalOutput")
        zs_d = nc.dram_tensor("z1s_s", [NP // NCORES, 64], F32, kind="ExternalOutput")
    else:
        src_in = nc.dram_tensor("z1s", [NP, 64], F32, kind="ExternalInput")
        zt_d = nc.dram_tensor("z2t_s", [NP // NCORES, 16], F32, kind="ExternalOutput")

    with tile.TileContext(nc) as tc:
        with (
            tc.tile_pool(name="pool", bufs=1) as pool,
            tc.tile_pool(name="gbuf", bufs=8) as gbuf,
            tc.tile_pool(name="dram", bufs=1, space="DRAM") as dram,
        ):
            nc.gpsimd.load_library(mlp)
            idx_sb = pool.tile([P, idx_cols], I16)
            nc.sync.dma_start(idx_sb[:], idx_d[:])
            dinvl = pool.tile([P, LPC], F32)
            nc.sync.dma_start(dinvl[:], dinv_d[:])

            if first:
                dinvw = pool.tile([P, NCOL], F32)
                nc.sync.dma_start(dinvw[:], dinvw_d[:])
                xw = pool.tile([P, NCOL, 16], F32)
                nc.sync.dma_start(xw[:], x_d.rearrange("(c p) f -> p c f", p=P))
                dinv2l = pool.tile([P, LPC], F32)
                nc.sync.dma_start(dinv2l[:], dinv2_d[:])
                src_h = dram.tile([NP, 64], F32)

            def body(_=None):
                if first:
                    # xs = x * dinv (full, replicated on every core), pad to 64
                    xs = pool.tile([P, NCOL, 64], F32, tag="xs")
                    nc.vector.memset(xs[:], 0.0)
                    nc.vector.tensor_tensor(
                        out=xs[:, :, 0:16], in0=xw[:],
                        in1=dinvw[:, :, None].to_broadcast([P, NCOL, 16]), op=MULT)
                    nc.sync.dma_start(src_h[:].rearrange("(c p) f -> p c f", p=P), xs[:])
                    src_ap = src_h[:]
                else:
                    src_ap = src_in[:]

                ctr = [0]
                acc = pool.tile([P, LPC, 64], F32, tag="acc")
                nc.vector.memset(acc[:], 0.0)
                _hop_gathers(nc, pool, gbuf, src_ap, idx_sb, tables["ngath"],
                             acc, 64, 64, F32, 1, ctr)

                # z*t = acc[:, :, :16] * dinv_local  -> [1280, 16]
                zt = pool.tile([P, LPC, 16], F32, tag="zt")
                nc.vector.tensor_tensor(
                    out=zt[:], in0=acc[:, :, 0:16],
                    in1=dinvl[:, :, None].to_broadcast([P, LPC, 16]), op=MULT)
                nc.sync.dma_start(zt_d.rearrange("(c p) f -> p c f", p=P), zt[:])

                if first:
                    zs = pool.tile([P, LPC, 64], F32, tag="zs")
                    nc.vector.tensor_tensor(
                        out=zs[:], in0=acc[:],
                        in1=dinv2l[:, :, None].to_broadcast([P, LPC, 64]), op=MULT)
                    nc.sync.dma_start(zs_d.rearrange("(c p) f -> p c f", p=P), zs[:])

            if reps == 1:
                body()
            else:
                with tc.For_i(0, reps, 1):
                    body()
    nc.compile()
    return nc


def _build_main(tables, reps=1):
    """Hin build (2 local cols) + layer-2 Horner with bf16 gather hops."""
    nc = bacc.Bacc(None, target_bir_lowering=False)
    idx_cols = tables["cidx"].shape[1]
    idx_d = nc.dram_tensor("idx", [P, idx_cols], I16, kind="ExternalInput")
    zin_d = nc.dram_tensor("zin", [NP, 8], F16, kind="ExternalInput")
    dinv_d = nc.dram_tensor("dinv", [P, NCOL], F32, kind="ExternalInput")
    dinv2_d = nc.dram_tensor("dinv2", [P, NCOL], F32, kind="ExternalInput")
    w1_d = nc.dram_tensor("w1", [3, HID], F16, kind="ExternalInput")
    b1_d = nc.dram_tensor("b1", [1, HID], F16, kind="ExternalInput")
    w212_d = nc.dram_tensor("w212", [P, 256], F16, kind="ExternalInput")  # blkdiag W2_1|W2_2
    w20_d = nc.dram_tensor("w20", [P, P], F16, kind="ExternalInput")      # blkdiag W2_0
    b2_d = nc.dram_tensor("b2", [1, HID], F32, kind="ExternalInput")
    h2_d = nc.dram_tensor("h2", [NP, P], F16, kind="ExternalOutput")

    from concourse.masks import make_identity
    with tile.TileContext(nc) as tc:
        with (
            tc.tile_pool(name="pool", bufs=1) as pool,
            tc.tile_pool(name="gbuf", bufs=10) as gbuf,
            tc.tile_pool(name="psA", bufs=2, space="PSUM") as psA,
            tc.tile_pool(name="psT", bufs=1, space="PSUM") as psT,
            tc.tile_pool(name="psH", bufs=4, space="PSUM") as psH,
            tc.tile_pool(name="dram", bufs=1, space="DRAM") as dram,
        ):
            nc.gpsimd.load_library(mlp)
            idx_sb = pool.tile([P, idx_cols], I16)
            nc.sync.dma_start(idx_sb[:], idx_d[:])
            dinv = pool.tile([P, NCOL], F32)
            nc.sync.dma_start(dinv[:], dinv_d[:])
            dinv2 = pool.tile([P, NCOL], F32)
            nc.sync.dma_start(dinv2[:], dinv2_d[:])
            zin = pool.tile([P, NCOL, 8], F16)
            nc.sync.dma_start(zin[:], zin_d.rearrange("(c p) f -> p c f", p=P))
            w1rep = pool.tile([P, 3, HID], F16)
            for k in range(3):
                nc.sync.dma_start(w1rep[:, k, :], w1_d[k:k + 1, :].to_broadcast([P, HID]))
            b1rep = pool.tile([P, HID], F16)
            nc.sync.dma_start(b1rep[:], b1_d[:].to_broadcast([P, HID]))
            b2rep = pool.tile([P, HID], F32)
            nc.sync.dma_start(b2rep[:], b2_d[:].to_broadcast([P, HID]))
            w212 = pool.tile([P, 256], F16)
            nc.sync.dma_start(w212[:], w212_d[:])
            w20 = pool.tile([P, P], F16)
            nc.sync.dma_start(w20[:], w20_d[:])
            ident = pool.tile([P, P], F32)
            make_identity(nc, ident[:])
            identb = pool.tile([P, P], F16)
            nc.vector.tensor_copy(out=identb[:], in_=ident[:])

            ws_h = dram.tile([NP, P], F16)
            vs_h = dram.tile([NP, P], F16)

            def body(_=None):
                # ---- Hin = relu(sum_k Zk[:,mloc] x W1_k + b1), both local cols ----
                Hin = pool.tile([P, NCOL, 2, HID], F16, tag="bigA")
                for mloc in range(2):
                    hm = Hin[:, :, mloc, :]
                    tmp = pool.tile([P, NCOL, HID], F16, tag="wv")
                    nc.vector.tensor_tensor(
                        out=hm, in0=zin[:, :, mloc:mloc + 1].to_broadcast([P, NCOL, HID]),
                        in1=w1rep[:, 0:1, :].to_broadcast([P, NCOL, HID]), op=MULT)
                    for k in (1, 2):
                        nc.vector.tensor_tensor(
                            out=tmp[:],
                            in0=zin[:, :, 2 * k + mloc:2 * k + mloc + 1].to_broadcast([P, NCOL, HID]),
                            in1=w1rep[:, k:k + 1, :].to_broadcast([P, NCOL, HID]), op=MULT)
                        nc.vector.tensor_tensor(out=hm, in0=hm, in1=tmp[:], op=ADD)
                    nc.vector.tensor_tensor(
                        out=hm, in0=hm,
                        in1=b1rep[:, None, :].to_broadcast([P, NCOL, HID]), op=ADD)
                    nc.vector.tensor_relu(out=hm, in_=hm)

                # ---- transpose Hin -> HinT [(2m x 64f), c, n] (f16) ----
                HinT = pool.tile([P, NCOL, P], F16, tag="bigC")
                for c in range(NCOL):
                    pt = psT.tile([P, P], F16, tag="pT")
                    nc.tensor.transpose(pt[:], Hin[:, c, :, :], identb[:])
                    nc.vector.tensor_copy(out=HinT[:, c, :], in_=pt[:])

                # ---- stage A: psum = [P1|P2] per (c); w = S P2 (bf16); P1 stays
                # resident in SBUF (bigA slot: Hin is dead after transposes) ----
                wv = pool.tile([P, NCOL, P], F16, tag="wv")
                p1sb = pool.tile([P, NCOL, 2, HID], F16, tag="bigA")
                for c in range(NCOL):
                    ps = psA.tile([P, 2, 2, HID], F32, tag="psA")
                    nc.tensor.matmul(ps[:].rearrange("p m k o -> p (m k o)"),
                                     HinT[:, c, :], w212[:], start=True, stop=True)
                    nc.vector.tensor_scalar_mul(
                        wv[:, c, :].rearrange("p (m o) -> p m o", m=2),
                        ps[:, :, 1, :], dinv[:, c:c + 1])
                    nc.vector.tensor_copy(out=p1sb[:, c, :, :], in_=ps[:, :, 0, :])
                nc.vector.memset(wv[:, NCOL - 1, :], 0.0)
                nc.sync.dma_start(ws_h[:].rearrange("(c p) f -> p c f", p=P), wv[:])

                # ---- P0 = Hin @ W2_0 early (PE overlaps the hop gathers) ----
                p0s = pool.tile([P, NCOL, P], F16, tag="p0s")
                for c in range(NCOL):
                    ps = psT.tile([P, 2, HID], F32, tag="p0")
                    nc.tensor.matmul(ps[:].rearrange("p m o -> p (m o)"),
                                     HinT[:, c, :], w20[:], start=True, stop=True)
                    nc.vector.tensor_copy(
                        out=p0s[:, c, :], in_=ps[:].rearrange("p m o -> p (m o)"))

                # ---- hop3: r = A_raw ws ----
                ctr = [0]
                racc = pool.tile([P, NCOL, P], F16, tag="bigB")
                _hop_gathers_psum(nc, gbuf, psH, ws_h[:], idx_sb,
                                  tables["cgath"], tables["clev"], racc,
                                  identb, ctr)

                # ---- v = S p1 + S^2 r (bf16 out), full-width ----
                p1v = p1sb[:].rearrange("p c m o -> p c (m o)")
                nc.vector.tensor_tensor(
                    out=p1v, in0=p1v,
                    in1=dinv[:, :, None].to_broadcast([P, NCOL, P]), op=MULT)
                nc.vector.tensor_tensor(
                    out=racc[:], in0=racc[:],
                    in1=dinv2[:, :, None].to_broadcast([P, NCOL, P]), op=MULT)
                wv2 = pool.tile([P, NCOL, P], F16, tag="wv")
                nc.vector.tensor_tensor(out=wv2[:], in0=racc[:], in1=p1v, op=ADD)
                nc.vector.memset(wv2[:, NCOL - 1, :], 0.0)
                nc.sync.dma_start(vs_h[:].rearrange("(c p) f -> p c f", p=P), wv2[:])

                # ---- hop4: s = A_raw vs (reuse racc slot) ----
                sacc = pool.tile([P, NCOL, P], F16, tag="bigB")
                _hop_gathers_psum(nc, gbuf, psH, vs_h[:], idx_sb,
                                  tables["cgath"], tables["clev"], sacc,
                                  identb, ctr)

                # ---- out2 = relu(P0 + S s + b2), full-width tail ----
                h2sb = pool.tile([P, NCOL, P], F16, tag="bigA")
                nc.vector.tensor_tensor(
                    out=sacc[:], in0=sacc[:],
                    in1=dinv[:, :, None].to_broadcast([P, NCOL, P]), op=MULT)
                nc.vector.tensor_tensor(out=h2sb[:], in0=p0s[:], in1=sacc[:], op=ADD)
                h2v = h2sb[:].rearrange("p c (m o) -> p c m o", m=2)
                nc.vector.tensor_tensor(
                    out=h2v, in0=h2v,
                    in1=b2rep[:, None, None, :].to_broadcast([P, NCOL, 2, HID]), op=ADD)
                nc.vector.tensor_relu(out=h2sb[:], in_=h2sb[:])
                nc.sync.dma_start(h2_d.rearrange("(c p) f -> p c f", p=P), h2sb[:])

            if reps == 1:
                body()
            else:
                with tc.For_i(0, reps, 1):
                    body()
    nc.compile()
    return nc


def _build_gru(reps=1):
    """GRU over T steps + head, feature-major, 2560 rows per core.

    All gates via tanh (one ACT table): sigmoid(x) = (1+tanh(x/2))/2, with
    the 1/2 factors folded algebraically:
      rt = tanh(0.5*pre_r + br/2); zt = tanh(0.5*pre_z + bz/2)
      u = pnh + b_hn;  n = tanh(pnx + b_in + 0.5*(u + rt*u))
      h' = n + 0.5*((h-n) + zt*(h-n))
    Gate matmuls bf16, x- and h-parts accumulated in PSUM.
    """
    nc = bacc.Bacc(None, target_bir_lowering=False)
    ROWS = B * NP // NCORES  # 2560
    CH = 512
    gx_d = nc.dram_tensor("gxb", [T, HID, ROWS], F16, kind="ExternalInput")
    wih_d = nc.dram_tensor("wihb", [HID, 3 * HID], F16, kind="ExternalInput")
    whh_d = nc.dram_tensor("whhb", [HID, 3 * HID], F16, kind="ExternalInput")
    # bias cols: 0=b_in, 1=b_hn, 2=br/2, 3=bz/2, 4=0.5
    bias_d = nc.dram_tensor("bias5", [HID, 5], F32, kind="ExternalInput")
    whead_d = nc.dram_tensor("whead", [HID, 1], F16, kind="ExternalInput")
    bhead_d = nc.dram_tensor("bhead", [1, 1], F32, kind="ExternalInput")
    y_d = nc.dram_tensor("y", [1, ROWS], F32, kind="ExternalOutput")

    SUB = mybir.AluOpType.subtract
    with tile.TileContext(nc) as tc:
        with (
            tc.tile_pool(name="pool", bufs=1) as pool,
            tc.tile_pool(name="ps", bufs=2, space="PSUM") as ps,
            tc.tile_pool(name="sb", bufs=3) as sb,
        ):
            gxs = pool.tile([HID, T, ROWS], F16)
            nc.sync.dma_start(gxs[:], gx_d.rearrange("t f n -> f t n"))
            wih = pool.tile([HID, 3 * HID], F16)
            nc.sync.dma_start(wih[:], wih_d[:])
            whh = pool.tile([HID, 3 * HID], F16)
            nc.sync.dma_start(whh[:], whh_d[:])
            bias = pool.tile([HID, 5], F32)
            nc.sync.dma_start(bias[:], bias_d[:])
            whead = pool.tile([HID, 1], F16)
            nc.sync.dma_start(whead[:], whead_d[:])
            bhead = pool.tile([1, 1], F32)
            nc.sync.dma_start(bhead[:], bhead_d[:])

            def body(_=None):
                h = pool.tile([HID, ROWS], F16)
                nc.vector.memset(h[:], 0.0)
                zt = pool.tile([HID, ROWS], F16, tag="zt")
                nn = pool.tile([HID, ROWS], F16, tag="nn")
                b_in, b_hn = bias[:, 0:1], bias[:, 1:2]
                br2, bz2 = bias[:, 2:3], bias[:, 3:4]

                for t in range(T):
                    for c0 in range(0, ROWS, CH):
                        sl = slice(c0, c0 + CH)
                        xt = gxs[:, t, sl]
                        hs = h[:, sl]
                        pr = ps.tile([HID, CH], F32, tag="pr")
                        pz = ps.tile([HID, CH], F32, tag="pz")
                        pnx = ps.tile([HID, CH], F32, tag="pnx")
                        pnh = ps.tile([HID, CH], F32, tag="pnh")
                        nc.tensor.matmul(pr[:], wih[:, 0:64], xt, start=True, stop=False)
                        nc.tensor.matmul(pr[:], whh[:, 0:64], hs, start=False, stop=True)
                        nc.tensor.matmul(pz[:], wih[:, 64:128], xt, start=True, stop=False)
                        nc.tensor.matmul(pz[:], whh[:, 64:128], hs, start=False, stop=True)
                        nc.tensor.matmul(pnx[:], wih[:, 128:192], xt, start=True, stop=True)
                        nc.tensor.matmul(pnh[:], whh[:, 128:192], hs, start=True, stop=True)
                        # rt = tanh(0.5*pr + br/2)
                        rt = sb.tile([HID, CH], F32, tag="rt")
                        nc.scalar.activation(rt[:], pr[:], AFT.Tanh,
                                             bias=br2, scale=0.5)
                        nc.scalar.activation(zt[:, sl], pz[:], AFT.Tanh,
                                             bias=bz2, scale=0.5)
                        # v1 = (pnh + b_hn) * rt ; v2 = (pnh + b_hn) + v1
                        v1 = sb.tile([HID, CH], F32, tag="v1")
                        nc.vector.scalar_tensor_tensor(
                            out=v1[:], in0=pnh[:], scalar=b_hn, in1=rt[:],
                            op0=ADD, op1=MULT)
                        v2 = sb.tile([HID, CH], F32, tag="v2")
                        nc.vector.scalar_tensor_tensor(
                            out=v2[:], in0=pnh[:], scalar=b_hn, in1=v1[:],
                            op0=ADD, op1=ADD)
                        # t2 = 0.5*v2 + pnx ; n = tanh(t2 + b_in)
                        t2 = sb.tile([HID, CH], F32, tag="t2")
                        nc.vector.scalar_tensor_tensor(
                            out=t2[:], in0=v2[:], scalar=0.5, in1=pnx[:],
                            op0=MULT, op1=ADD)
                        nc.scalar.activation(nn[:, sl], t2[:], AFT.Tanh,
                                             bias=b_in)
                    # h' = n + 0.5*((h-n) + zt*(h-n)), full width (f16, 2x DVE)
                    hm = sb.tile([HID, ROWS], F16, tag="hm")
                    nc.vector.tensor_tensor(out=hm[:], in0=h[:], in1=nn[:], op=SUB)
                    hm2 = sb.tile([HID, ROWS], F16, tag="hm2")
                    nc.vector.tensor_tensor(out=hm2[:], in0=zt[:], in1=hm[:], op=MULT)
                    nc.vector.tensor_tensor(out=hm[:], in0=hm[:], in1=hm2[:], op=ADD)
                    nc.vector.scalar_tensor_tensor(
                        out=h[:], in0=hm[:], scalar=0.5, in1=nn[:],
                        op0=MULT, op1=ADD)

                ysb = pool.tile([1, ROWS], F32)
                for c0 in range(0, ROWS, CH):
                    py_full = ps.tile([HID, CH], F32, tag="pr")
                    py = py_full[:1, :]
                    nc.tensor.matmul(py[:], whead[:], h[:, c0:c0 + CH], start=True, stop=True)
                    nc.vector.tensor_scalar_add(ysb[:, c0:c0 + CH], py[:], bhead[:])
                nc.sync.dma_start(y_d[:], ysb[:])

            if reps == 1:
                body()
            else:
                with tc.For_i(0, reps, 1):
                    body()
    nc.compile()
    return nc


# ----------------------------------------------------------------------------
# top-level kernel
# ----------------------------------------------------------------------------

def _get_programs(edge_src, edge_dst):
    reps = TIMING_REPS
    h = (hash(edge_src.tobytes()), hash(edge_dst.tobytes()), reps)
    if h not in _CACHE:
        tables = _CACHE.get(("tables", h[0], h[1]))
        if tables is None:
            tables = _prep(edge_src, edge_dst)
            _CACHE[("tables", h[0], h[1])] = tables
        _CACHE[h] = dict(
            tables=tables,
            A0=_build_hop_node(tables, True, reps),
            A1=_build_hop_node(tables, False, reps),
            AM=_build_main(tables, reps),
            GB=_build_gru(reps),
        )
    return _CACHE[h]


def _run(progs, name, in_maps, cores):
    import time
    t0 = time.time()
    r = run_bass_kernel_spmd(progs[name], in_maps, cores)
    LAST_WALLS[name] = time.time() - t0
    return r.results


def kernel(x, edge_src, edge_dst, edge_val,
           W_sp1, b_sp1, W_sp2, b_sp2,
           W_ih, W_hh, b_ih, b_hh, W_head, b_head):
    x = np.asarray(x, np.float32)
    edge_src = np.asarray(edge_src, np.int32)
    edge_dst = np.asarray(edge_dst, np.int32)
    W_sp1 = np.asarray(W_sp1, np.float32)
    b_sp1 = np.asarray(b_sp1, np.float32)
    W_sp2 = np.asarray(W_sp2, np.float32)
    b_sp2 = np.asarray(b_sp2, np.float32)
    W_ih = np.asarray(W_ih, np.float32)
    W_hh = np.asarray(W_hh, np.float32)
    b_ih = np.asarray(b_ih, np.float32)
    b_hh = np.asarray(b_hh, np.float32)
    W_head = np.asarray(W_head, np.float32)
    b_head = np.asarray(b_head, np.float32)

    progs = _get_programs(edge_src, edge_dst)
    tables = progs["tables"]
    order = tables["order"]
    dinv_n = tables["dinv_n"]
    cores = list(range(NCORES))

    # x in new-id space, [NP, 16]
    x16 = np.zeros((NP, M), np.float32)
    x16[:N] = x.transpose(1, 0, 2).reshape(N, M)[order]

    dinvw = _wrap(dinv_n)
    # ---- launch A0: hop1 ----
    in0 = []
    for c in cores:
        nodes = tables["node_of"][c]
        dl = dinv_n[nodes]
        in0.append({
            "x16": x16, "idxn": tables["nidx"][c],
            "dinvl": np.ascontiguousarray(dl.reshape(LPC, P).T),
            "dinv2l": np.ascontiguousarray((dl * dl).reshape(LPC, P).T),
            "dinvw": dinvw,
        })
    r0 = _run(progs, "A0", in0, cores)
    z1t = np.zeros((NP, 16), np.float32)
    z1s = np.zeros((NP, 64), np.float32)
    for c in cores:
        z1t[tables["node_of"][c]] = r0[c]["z1t_s"]
        z1s[tables["node_of"][c]] = r0[c]["z1s_s"]

    # ---- launch A1: hop2 ----
    in1 = []
    for c in cores:
        nodes = tables["node_of"][c]
        dl = dinv_n[nodes]
        in1.append({
            "z1s": z1s, "idxn": tables["nidx"][c],
            "dinvl": np.ascontiguousarray(dl.reshape(LPC, P).T),
            "dinv2l": np.ascontiguousarray((dl * dl).reshape(LPC, P).T),
        })
    r1 = _run(progs, "A1", in1, cores)
    z2t = np.zeros((NP, 16), np.float32)
    for c in cores:
        z2t[tables["node_of"][c]] = r1[c]["z2t_s"]

    # ---- launch A-main: Hin + layer 2 ----
    W1 = W_sp1[:, 0, :]                      # [3, HID]
    w212 = np.zeros((P, 256), np.float32)    # block-diag [W2_1|W2_2] per mloc
    w20 = np.zeros((P, P), np.float32)
    for m2 in range(2):
        w212[64 * m2:64 * m2 + 64, 128 * m2:128 * m2 + 64] = W_sp2[1]
        w212[64 * m2:64 * m2 + 64, 128 * m2 + 64:128 * m2 + 128] = W_sp2[2]
        w20[64 * m2:64 * m2 + 64, 64 * m2:64 * m2 + 64] = W_sp2[0]
    inm = []
    for c in cores:
        m0, m1 = 2 * c, 2 * c + 1
        zin = np.zeros((NP, 8), np.float32)
        zin[:, 0] = x16[:, m0]
        zin[:, 1] = x16[:, m1]
        zin[:, 2] = z1t[:, m0]
        zin[:, 3] = z1t[:, m1]
        zin[:, 4] = z2t[:, m0]
        zin[:, 5] = z2t[:, m1]
        inm.append({
            "idx": tables["cidx"], "zin": zin.astype(np.float16),
            "dinv": dinvw, "dinv2": dinvw * dinvw,
            "w1": W1.astype(np.float16), "b1": b_sp1[None, :].astype(np.float16),
            "w212": w212.astype(np.float16), "w20": w20.astype(np.float16),
            "b2": b_sp2[None, :],
        })
    rm = _run(progs, "AM", inm, cores)
    H2 = np.zeros((NP, M, HID), np.float32)
    for c in cores:
        h2c = rm[c]["h2"].reshape(NP, 2, HID)
        H2[:, 2 * c, :] = h2c[:, 0, :]
        H2[:, 2 * c + 1, :] = h2c[:, 1, :]

    # ---- launch B: GRU + head ----
    ROWS = B * NP // NCORES
    # sequences feature-major: seq[t, f, b*NP + rank]
    seq = H2.reshape(NP, B, T, HID).transpose(2, 3, 1, 0).reshape(T, HID, B * NP)
    bias5 = np.stack([
        b_ih[128:192],
        b_hh[128:192],
        (b_ih[0:64] + b_hh[0:64]) * 0.5,
        (b_ih[64:128] + b_hh[64:128]) * 0.5,
        np.full(64, 0.5, np.float32),
    ], axis=1).astype(np.float32)            # [64, 5]
    wihb = np.ascontiguousarray(W_ih.T).astype(np.float16)
    whhb = np.ascontiguousarray(W_hh.T).astype(np.float16)
    inb = []
    for c in cores:
        inb.append({
            "gxb": np.ascontiguousarray(
                seq[:, :, c * ROWS:(c + 1) * ROWS]).astype(np.float16),
            "wihb": wihb, "whhb": whhb,
            "bias5": bias5, "whead": W_head.astype(np.float16),
            "bhead": b_head[None, :],
        })
    rb = _run(progs, "GB", inb, cores)
    y_new = np.concatenate([rb[c]["y"][0] for c in cores]).reshape(B, NP)
    y = y_new[:, tables["perm"]]
    return np.ascontiguousarray(y.astype(np.float32))

